# revision 4
# baseline (speedup 1.0000x reference)
"""Trainium2 Bass kernel for nn_MinimalAttention (B=1, S=4096, d_model=768,
H=12, Dh=64, post-softmax causal mask).

Sharding (8 cores): 4 head-groups (3 heads each) x 2 sequence shards.
Each seq shard owns 2048 query rows as 16 128-row subtiles, mod-4
interleaved across the sequence so the causal-mask work pattern is
identical on every core (the SPMD program is branch-free; all per-core
variation is input data: weight slices, pre-gathered xTq columns, mask
tiles).

Per core:
  K^T/Q^T projections in float32r (full-rate fp32 storage), V projection
  in bf16; scores^T = K_h^T slice x Q_h^T per 128-key tile (bf16, PSUM
  f32); exp on ScalarE writing bf16 E tiles; y accumulation as
  y[q,0:64] += E_kt^T @ V_kt with a fused ones column (y[q,64] becomes
  the full softmax denominator; diagonal-window tiles use host-provided
  M / 1-M masks so numerator masking and denominator completeness both
  hold); per-partition reciprocal + scale; PE transpose to y^T; partial
  output projection outT = W_out_slice^T chunk @ y^T.

Host sums the 4 head-group partials per shard, adds b_out, and scatters
the interleaved rows back.
"""
import sys

sys.path.insert(0, "/opt/trn_rl_repo")

import numpy as np
import ml_dtypes

S, D, H, DH = 4096, 768, 12, 64
N_CORES = 8
GD = 192          # head-group dims (3 heads)
LT = 16           # local 128-row subtiles per core (2048 q rows)
NK = 32           # key tiles

_cache = {}


def _g_of(s, t):
    k = t // 2
    if s == 0:
        return 4 * k + (0 if t % 2 == 0 else 3)
    return 4 * k + (1 if t % 2 == 0 else 2)


def _qcols(s):
    idx = []
    for t in range(LT):
        g = _g_of(s, t)
        idx.extend(range(g * 128, g * 128 + 128))
    return np.array(idx)


def _masks(s):
    M = np.zeros((8, 128, 128), np.float32)
    for p in (0, 1):
        delta = _g_of(s, p)  # k4 == 0 for t in (0,1)
        for j in range(4):
            if j < delta:
                M[p * 4 + j] = 1.0
            elif j == delta:
                M[p * 4 + j] = (
                    np.arange(128)[:, None] <= np.arange(128)[None, :]
                ).astype(np.float32)
    return M, 1.0 - M


def _build_program():
    import concourse.bass as bass
    import concourse.mybir as mybir
    import concourse.tile as tile
    from concourse import bacc

    f32 = mybir.dt.float32
    f32r = mybir.dt.float32r
    bf16 = mybir.dt.bfloat16
    Exp = mybir.ActivationFunctionType.Exp
    mult = mybir.AluOpType.mult
    add = mybir.AluOpType.add

    nc = bacc.Bacc(
        "TRN2",
        target_bir_lowering=False,
        debug=False,
        enable_asserts=False,
        num_devices=N_CORES,
    )

    d_xT = nc.dram_tensor("xt_in", [D, S], f32r, kind="ExternalInput").ap()
    d_xTq = nc.dram_tensor("xtq_in", [D, 2048], f32r, kind="ExternalInput").ap()
    d_wq = nc.dram_tensor("wq_in", [D, GD], f32r, kind="ExternalInput").ap()
    d_wk = nc.dram_tensor("wk_in", [D, GD], f32r, kind="ExternalInput").ap()
    d_wv = nc.dram_tensor("wv_in", [D, GD], bf16, kind="ExternalInput").ap()
    d_bq = nc.dram_tensor("bq_in", [GD, 1], f32, kind="ExternalInput").ap()
    d_bk = nc.dram_tensor("bk_in", [GD, 1], f32, kind="ExternalInput").ap()
    d_bvb = nc.dram_tensor("bvb_in", [128, GD], f32, kind="ExternalInput").ap()
    d_wo = nc.dram_tensor("wo_in", [GD, D], bf16, kind="ExternalInput").ap()
    d_mm = nc.dram_tensor("mm_in", [8, 128, 128], bf16, kind="ExternalInput").ap()
    d_mc = nc.dram_tensor("mc_in", [8, 128, 128], bf16, kind="ExternalInput").ap()
    d_id = nc.dram_tensor("id_in", [128, 128], bf16, kind="ExternalInput").ap()
    d_out = nc.dram_tensor("outt_out", [D, 2048], f32, kind="ExternalOutput").ap()

    def r(ap):
        return ap.bitcast(f32r)

    with tile.TileContext(nc) as tc:
        with tc.tile_pool(name="const", bufs=1) as cp:
            wq_sb = [cp.tile([128, GD], f32r, tag=f"wq{k}", name=f"wq{k}") for k in range(6)]
            wk_sb = [cp.tile([128, GD], f32r, tag=f"wk{k}", name=f"wk{k}") for k in range(6)]
            wv_sb = [cp.tile([128, GD], bf16, tag=f"wv{k}", name=f"wv{k}") for k in range(6)]
            wo0 = cp.tile([128, D], bf16, tag="wo0")
            wo1 = cp.tile([64, D], bf16, tag="wo1")
            bq0 = cp.tile([128, 1], f32, tag="bq0")
            bq1 = cp.tile([64, 1], f32, tag="bq1")
            bk0 = cp.tile([128, 1], f32, tag="bk0")
            bk1 = cp.tile([64, 1], f32, tag="bk1")
            bvb = cp.tile([128, GD], f32, tag="bvb")
            mm_sb = cp.tile([128, 8, 128], bf16, tag="mm")
            mc_sb = cp.tile([128, 8, 128], bf16, tag="mc")
            id_sb = cp.tile([128, 128], bf16, tag="ident")
            ones = cp.tile([128, 1], bf16, tag="ones")
            KT0 = cp.tile([128, S], bf16, tag="KT0")
            KT1 = cp.tile([64, S], bf16, tag="KT1")
            QT0 = cp.tile([128, 2048], bf16, tag="QT0")
            QT1 = cp.tile([64, 2048], bf16, tag="QT1")
            Vb = cp.tile([128, 3, NK, 65], bf16, tag="Vb")
            yT0 = cp.tile([128, 2048], bf16, tag="yT0")
            yT1 = cp.tile([64, 2048], bf16, tag="yT1")

            for k in range(6):
                nc.sync.dma_start(out=wq_sb[k][:], in_=d_wq[k * 128:(k + 1) * 128, :])
                nc.sync.dma_start(out=wk_sb[k][:], in_=d_wk[k * 128:(k + 1) * 128, :])
                nc.sync.dma_start(out=wv_sb[k][:], in_=d_wv[k * 128:(k + 1) * 128, :])
            nc.sync.dma_start(out=wo0[:], in_=d_wo[0:128, :])
            nc.sync.dma_start(out=wo1[:], in_=d_wo[128:GD, :])
            nc.sync.dma_start(out=bq0[:], in_=d_bq[0:128, :])
            nc.sync.dma_start(out=bq1[:], in_=d_bq[128:GD, :])
            nc.sync.dma_start(out=bk0[:], in_=d_bk[0:128, :])
            nc.sync.dma_start(out=bk1[:], in_=d_bk[128:GD, :])
            nc.sync.dma_start(out=bvb[:], in_=d_bvb[:, :])
            for m in range(8):
                nc.sync.dma_start(out=mm_sb[:, m, :], in_=d_mm[m, :, :])
                nc.sync.dma_start(out=mc_sb[:, m, :], in_=d_mc[m, :, :])
            nc.sync.dma_start(out=id_sb[:], in_=d_id[:, :])
            nc.vector.memset(ones[:], 1.0)
            nc.vector.memset(Vb[:, :, :, 64:65], 1.0)

            # ---------------- projections ----------------
            with (
                tc.tile_pool(name="xsl", bufs=14) as xp,
                tc.tile_pool(name="xbf", bufs=14) as xbp,
                tc.tile_pool(name="psP", bufs=4, space="PSUM") as pp,
            ):
                # Q^T = wq^T @ xTq  (+bq), cast bf16
                for qb in range(4):
                    xq = []
                    for k in range(6):
                        tl_ = xp.tile([128, 512], f32r, tag="xsl")
                        nc.sync.dma_start(
                            out=tl_[:],
                            in_=d_xTq[k * 128:(k + 1) * 128, qb * 512:(qb + 1) * 512],
                        )
                        xq.append(tl_)
                    for msz, off, QT_t, bq_t in ((128, 0, QT0, bq0), (64, 128, QT1, bq1)):
                        ps = pp.tile([msz, 512], f32, tag="psP")
                        for k in range(6):
                            nc.tensor.matmul(
                                ps[:],
                                wq_sb[k][:, off:off + msz],
                                xq[k][:],
                                start=(k == 0),
                                stop=(k == 5),
                            )
                        nc.vector.tensor_scalar_add(
                            QT_t[:, qb * 512:(qb + 1) * 512], ps[:], bq_t[:]
                        )
                # K^T (+bk) and V (natural, bf16, +bv)
                for nb in range(8):
                    xs = []
                    xb = []
                    for k in range(6):
                        tl_ = xp.tile([128, 512], f32r, tag="xsl")
                        nc.sync.dma_start(
                            out=tl_[:],
                            in_=d_xT[k * 128:(k + 1) * 128, nb * 512:(nb + 1) * 512],
                        )
                        xs.append(tl_)
                        tb = xbp.tile([128, 512], bf16, tag="xbf")
                        nc.vector.tensor_copy(tb[:], tl_[:].bitcast(f32))
                        xb.append(tb)
                    for msz, off, KT_t, bk_t in ((128, 0, KT0, bk0), (64, 128, KT1, bk1)):
                        ps = pp.tile([msz, 512], f32, tag="psP")
                        for k in range(6):
                            nc.tensor.matmul(
                                ps[:],
                                wk_sb[k][:, off:off + msz],
                                xs[k][:],
                                start=(k == 0),
                                stop=(k == 5),
                            )
                        nc.vector.tensor_scalar_add(
                            KT_t[:, nb * 512:(nb + 1) * 512], ps[:], bk_t[:]
                        )
                    for ms in range(4):
                        kt = nb * 4 + ms
                        ps = pp.tile([128, GD], f32, tag="psP")
                        for k in range(6):
                            nc.tensor.matmul(
                                ps[:],
                                xb[k][:, ms * 128:(ms + 1) * 128],
                                wv_sb[k][:],
                                start=(k == 0),
                                stop=(k == 5),
                            )
                        for h in range(3):
                            nc.vector.tensor_tensor(
                                Vb[:, h, kt, 0:64],
                                ps[:, h * 64:(h + 1) * 64],
                                bvb[:, h * 64:(h + 1) * 64],
                                add,
                            )

            # ---------------- attention ----------------
            with (
                tc.tile_pool(name="psA", bufs=1, space="PSUM") as pa,
                tc.tile_pool(name="psB", bufs=1, space="PSUM") as pb,
                tc.tile_pool(name="psY", bufs=1, space="PSUM") as py,
                tc.tile_pool(name="psT", bufs=1, space="PSUM") as pt,
                tc.tile_pool(name="epool", bufs=2) as ep,
                tc.tile_pool(name="small", bufs=3) as sp,
                tc.tile_pool(name="ocp", bufs=3) as op_,
            ):
                groups = [
                    (0, 4), (4, 2), (6, 4), (10, 2), (12, 4), (16, 2),
                    (18, 4), (22, 2), (24, 4), (28, 2), (30, 2),
                ]
                for h in range(3):
                    if h < 2:
                        KTh = KT0[64 * h:64 * (h + 1), :]
                        QTh = QT0[64 * h:64 * (h + 1), :]
                    else:
                        KTh = KT1[0:64, :]
                        QTh = QT1[0:64, :]
                    for qb in range(4):
                        E = ep.tile([128, NK, 512], bf16, tag="E")
                        for k0, gsz in groups:
                            pool = pa if gsz == 4 else pb
                            tag = "psA" if gsz == 4 else "psB"
                            ps = pool.tile([128, gsz * 512], f32, tag=tag)
                            for i in range(gsz):
                                kt = k0 + i
                                nc.tensor.matmul(
                                    ps[:, i * 512:(i + 1) * 512],
                                    KTh[:, kt * 128:(kt + 1) * 128],
                                    QTh[:, qb * 512:(qb + 1) * 512],
                                    start=True,
                                    stop=True,
                                )
                            nc.scalar.activation(
                                E[:, k0:k0 + gsz, :], ps[:, 0:gsz * 512], Exp
                            )
                        for tl_i in range(4):
                            t = qb * 4 + tl_i
                            k4 = 4 * (t // 2)
                            p = t % 2
                            tsl = slice(tl_i * 128, (tl_i + 1) * 128)
                            yp = py.tile([128, 65], f32, tag="psY")
                            first = True
                            for kt in range(k4):
                                nc.tensor.matmul(
                                    yp[:],
                                    E[:, kt, tsl],
                                    Vb[:, h, kt, :],
                                    start=first,
                                    stop=False,
                                )
                                first = False
                            for j in range(4):
                                kt = k4 + j
                                em = sp.tile([128, 128], bf16, tag="em")
                                nc.vector.tensor_tensor(
                                    em[:], E[:, kt, tsl], mm_sb[:, p * 4 + j, :], mult
                                )
                                nc.tensor.matmul(
                                    yp[:], em[:], Vb[:, h, kt, :],
                                    start=first, stop=False,
                                )
                                first = False
                                ec = sp.tile([128, 128], bf16, tag="ec")
                                nc.vector.tensor_tensor(
                                    ec[:], E[:, kt, tsl], mc_sb[:, p * 4 + j, :], mult
                                )
                                nc.tensor.matmul(
                                    yp[:, 64:65], ec[:], ones[:],
                                    start=False, stop=False,
                                )
                            for kt in range(k4 + 4, NK):
                                nc.tensor.matmul(
                                    yp[:, 64:65],
                                    E[:, kt, tsl],
                                    ones[:],
                                    start=False,
                                    stop=(kt == NK - 1),
                                )
                            rc = sp.tile([128, 1], f32, tag="rc")
                            nc.vector.reciprocal(rc[:], yp[:, 64:65])
                            ysb = sp.tile([128, 64], bf16, tag="ysb")
                            nc.vector.tensor_scalar_mul(ysb[:], yp[:, 0:64], rc[:])
                            tp = pt.tile([64, 128], bf16, tag="psT")
                            nc.tensor.transpose(tp[:], ysb[:], id_sb[:])
                            if h < 2:
                                dst = yT0[64 * h:64 * (h + 1), t * 128:(t + 1) * 128]
                            else:
                                dst = yT1[0:64, t * 128:(t + 1) * 128]
                            nc.vector.tensor_copy(dst, tp[:])

                # ---------------- output projection ----------------
                for mt in range(6):
                    for qb in range(4):
                        ps = pb.tile([128, 512], f32, tag="psB")
                        nc.tensor.matmul(
                            ps[:],
                            wo0[:, mt * 128:(mt + 1) * 128],
                            yT0[:, qb * 512:(qb + 1) * 512],
                            start=True,
                            stop=False,
                        )
                        nc.tensor.matmul(
                            ps[:],
                            wo1[:, mt * 128:(mt + 1) * 128],
                            yT1[:, qb * 512:(qb + 1) * 512],
                            start=False,
                            stop=True,
                        )
                        oc = op_.tile([128, 512], f32, tag="ocp")
                        nc.vector.tensor_copy(oc[:], ps[:])
                        nc.sync.dma_start(
                            out=d_out[mt * 128:(mt + 1) * 128, qb * 512:(qb + 1) * 512],
                            in_=oc[:],
                        )

    nc.compile()
    return nc


def _get_program():
    if "nc" not in _cache:
        _cache["nc"] = _build_program()
    return _cache["nc"]


def shard_inputs(x, W_qkv, b_qkv, W_out, b_out):
    """Build the 8 per-core input maps."""
    bf = ml_dtypes.bfloat16
    xT = np.ascontiguousarray(x[0].T.astype(np.float32))  # [D, S]
    ident = np.eye(128, dtype=np.float32).astype(bf)
    in_maps = []
    per_s = {}
    for s in (0, 1):
        cols = _qcols(s)
        M, Mc = _masks(s)
        per_s[s] = (
            np.ascontiguousarray(xT[:, cols]),
            M.astype(bf),
            Mc.astype(bf),
        )
    for c in range(N_CORES):
        hg, s = c // 2, c % 2
        hsl = slice(GD * hg, GD * (hg + 1))
        xTq, M, Mc = per_s[s]
        wq = np.ascontiguousarray((W_qkv[0:768][hsl] / 8.0).T.astype(np.float32))
        wk = np.ascontiguousarray(W_qkv[768:1536][hsl].T.astype(np.float32))
        wv = np.ascontiguousarray(W_qkv[1536:2304][hsl].T.astype(np.float32)).astype(bf)
        bq = (b_qkv[0:768][hsl] / 8.0).astype(np.float32).reshape(GD, 1)
        bk = b_qkv[768:1536][hsl].astype(np.float32).reshape(GD, 1)
        bv = b_qkv[1536:2304][hsl].astype(np.float32)
        bvb = np.ascontiguousarray(np.broadcast_to(bv[None, :], (128, GD)))
        wo = np.ascontiguousarray(W_out[:, hsl].T.astype(np.float32)).astype(bf)
        in_maps.append(
            {
                "xt_in": xT,
                "xtq_in": xTq,
                "wq_in": wq,
                "wk_in": wk,
                "wv_in": wv,
                "bq_in": bq,
                "bk_in": bk,
                "bvb_in": bvb,
                "wo_in": wo,
                "mm_in": M,
                "mc_in": Mc,
                "id_in": ident,
            }
        )
    return in_maps


def gather_output(results, b_out):
    out = np.zeros((S, D), np.float32)
    for s in (0, 1):
        acc = np.zeros((2048, D), np.float32)
        for hg in range(4):
            c = hg * 2 + s
            acc += results[c]["outt_out"].T
        out[_qcols(s)] = acc + b_out[None, :].astype(np.float32)
    return out.reshape(1, S, D)


def kernel(x, W_qkv, b_qkv, W_out, b_out):
    from concourse.bass_utils import run_bass_kernel_spmd

    x = np.asarray(x)
    W_qkv = np.asarray(W_qkv)
    b_qkv = np.asarray(b_qkv)
    W_out = np.asarray(W_out)
    b_out = np.asarray(b_out)
    nc = _get_program()
    in_maps = shard_inputs(x, W_qkv, b_qkv, W_out, b_out)
    res = run_bass_kernel_spmd(nc, in_maps, list(range(N_CORES)))
    return gather_output(res.results, b_out)
